# revision 72
# baseline (speedup 1.0000x reference)
"""Trainium2 Bass kernel for nn_DecNP (two-stage KNN feature propagation).

Algorithm (per stage): rank coarse points per query with a PE matmul
(coords split bf16 hi/lo so a contraction-16 bf16 matmul reproduces the
fp32 ranking to ~2^-17), top-8 via DVE max8/find_index8, one batched
indirect DMA per query tile to gather neighbour meta rows (xyz fp32,
percentages/directions bf16), direction-mask weights on DVE with the
unnormalized-direction compare |d.v| > g*(dist+eps)*|d|, feature gather
(stage 0: merged row, stage 1: straight from the AllGather output), PE
diagonal-weight interpolation, fused residual + L2 normalize.

Sharding: query rows split across 8 cores.  Stage-0 output is
AllGather'd (it is the feature table of stage 1); the scalar mean of
de_k_weight_sum is AllReduce'd per stage, launched as soon as the last
weight tile finishes so the finalize never waits on it.
"""
import sys

for _p in ("/opt/trn_rl_repo", "/root/.axon_site/_ro/trn_rl_repo", "/root/.axon_site"):
    if _p not in sys.path:
        sys.path.append(_p)

import numpy as np
import ml_dtypes

import concourse.bacc as bacc
import concourse.bass as bass
import concourse.bass_isa as bass_isa
import concourse.mybir as mybir
from concourse.masks import make_identity
from concourse.tile import TileContext

NCORES = 8
P = 128
D = 768
K = 8
M = 20
GAMMA = 0.85
EPS_DIR = 1e-8
BF16 = mybir.dt.bfloat16
F32 = mybir.dt.float32
U32 = mybir.dt.uint32
U16 = mybir.dt.uint16
I16 = mybir.dt.int16
X = mybir.AxisListType.X
Copy = mybir.ActivationFunctionType.Copy
Sqrt = mybir.ActivationFunctionType.Sqrt
Square = mybir.ActivationFunctionType.Square
Abs = mybir.ActivationFunctionType.Abs
Mult = mybir.AluOpType.mult
Add = mybir.AluOpType.add
Sub = mybir.AluOpType.subtract
IsGt = mybir.AluOpType.is_gt

# table row layouts, in fp32 words
# t0 (stage 0, merged): 0:3 xyz | 4:14 perc bf16 | 14:44 dirs bf16 | 64:448 feat bf16
# t1m (stage 1, meta):  0:3 xyz | 4:14 perc bf16 | 14:44 dirs bf16 | pad to 64
T0W = 448
T1W = 64

ST0 = dict(S=1024, Q=512, NT=4096)
ST1 = dict(S=4096, Q=2048, NT=16384)
C_SCAL = 0.3  # N == 4*S in both stages

RG = [list(range(NCORES))]

_CACHE = {}


class Stage:
    def __init__(self, nc, pools, identF, identB, *, st, S, Q, NT, tbl, tw,
                 qxyz, p1, out_rows, sum_in, sum_out):
        self.__dict__.update(locals())
        self.n_st = S // P
        self.n_qt = Q // P
        self.tba = tbl.ap()
        self.p1a = p1.ap()
        self.ora = out_rows.ap()

    def emit_tables(self):
        """c8 = [chi;clo;chi;clo] rows (x,y,z,|s|^2 hi/lo bf16), q8 likewise."""
        nc, pools = self.nc, self.pools
        st, S, Q = self.st, self.S, self.Q
        c8 = pools["tbl"].tile([16, S], BF16, tag=f"c8_{st}")
        self.c8 = c8
        for i in range(self.n_st):
            rs = slice(i * P, (i + 1) * P)
            xyz = pools["mw"].tile([P, 3], F32, tag="xyz")
            nc.sync.dma_start(out=xyz[:, :], in_=self.tba[rs, 0:3])
            ca = pools["mw"].tile([P, 4], F32, tag="ca")
            nc.vector.tensor_copy(ca[:, 0:3], xyz[:, :])
            sq3 = pools["mw"].tile([P, 3], F32, tag="sq3")
            nc.vector.tensor_mul(sq3[:, :], xyz[:, :], xyz[:, :])
            nc.vector.reduce_sum(out=ca[:, 3:4], in_=sq3[:, :], axis=X)
            # hl = [chi | clo | chi | clo] in the free dim, then one transpose
            hl = pools["mw"].tile([P, 16], BF16, tag="hl")
            nc.scalar.activation(out=hl[:, 0:4], in_=ca[:, :], func=Copy)
            nc.vector.tensor_tensor(out=hl[:, 4:8], in0=ca[:, :],
                                    in1=hl[:, 0:4], op=Sub)
            nc.vector.tensor_copy(hl[:, 8:16], hl[:, 0:8])
            ptp = pools["pt"].tile([16, P], BF16, tag="ptp")
            nc.tensor.transpose(out=ptp[:, :], in_=hl[:, :],
                                identity=self.identB[:, :])
            nc.scalar.activation(out=c8[:, rs], in_=ptp[:, :], func=Copy)

        qxall = pools["tbl"].tile([P, self.n_qt, 3], F32, tag=f"qxall_{st}")
        self.qxall = qxall
        q8 = pools["tbl"].tile([16, Q], BF16, tag=f"q8_{st}")
        self.q8 = q8
        qxa = self.qxyz.ap()
        for t in range(self.n_qt):
            rs = slice(t * P, (t + 1) * P)
            nc.sync.dma_start(out=qxall[:, t, :], in_=qxa[rs, :])
            qt4 = pools["mw"].tile([P, 4], F32, tag="qt4")
            nc.vector.tensor_scalar_mul(qt4[:, 0:3], qxall[:, t, :], 2.0)
            nc.vector.memset(qt4[:, 3:4], -1.0)
            # hl = [qhi | qhi | qlo | qlo] in the free dim, then one transpose
            hl = pools["mw"].tile([P, 16], BF16, tag="hl")
            nc.scalar.activation(out=hl[:, 0:4], in_=qt4[:, :], func=Copy)
            nc.vector.tensor_copy(hl[:, 4:8], hl[:, 0:4])
            nc.vector.tensor_tensor(out=hl[:, 8:12], in0=qt4[:, :],
                                    in1=hl[:, 0:4], op=Sub)
            nc.vector.tensor_copy(hl[:, 12:16], hl[:, 8:12])
            ptp = pools["pt"].tile([16, P], BF16, tag="ptp")
            nc.tensor.transpose(out=ptp[:, :], in_=hl[:, :],
                                identity=self.identB[:, :])
            nc.scalar.activation(out=q8[:, rs], in_=ptp[:, :], func=Copy)

        # wrapped dma_gather index tiles: [16 partitions x 64] per query tile,
        # replicated to all 8 gpsimd-core stripes (128 partitions total)
        self.widx = pools["tbl"].tile([P, self.n_qt, 64], I16,
                                      tag=f"widx_{st}")
        self.acc = pools["tbl"].tile([P, 1], F32, tag=f"acc_{st}")
        nc.vector.memset(self.acc[:, :], 0.0)
        self.gms = {}

    def emit_rank(self, t):
        """negE = 2q.c - |c|^2 via bf16 hi/lo matmul; top-8 indices."""
        nc, pools = self.nc, self.pools
        S = self.S
        rs = slice(t * P, (t + 1) * P)
        negE = pools["neg"].tile([P, S], F32, tag="negE")
        for c in range(S // 512):
            pe = pools["pe"].tile([P, 512], F32, tag="pe")
            nc.tensor.matmul(out=pe[:, :], lhsT=self.q8[:, rs],
                             rhs=self.c8[:, c * 512:(c + 1) * 512],
                             start=True, stop=True)
            nc.scalar.activation(out=negE[:, c * 512:(c + 1) * 512],
                                 in_=pe[:, :], func=Copy)
        best = pools["wk"].tile([P, 8], F32, tag="best")
        ix = pools["ix"].tile([P, K], U16, tag="ix")
        nc.vector.max(out=best[:, :], in_=negE[:, :])
        nc.vector.max_index(out=ix[:, :], in_max=best[:, :],
                            in_values=negE[:, :])
        nc.sync.dma_start(out=self.dixa[t].ap(), in_=ix[:, :].bitcast(I16))
        # build the wrapped dma_gather index layout, widx[r, 8k+c] =
        # idx[16c+r, k], via a DRAM bounce (contiguous 16B runs) + an
        # on-chip free-dim transpose + log-doubling partition replication
        tw = pools["ix"].tile([16, 64], I16, tag="tw")
        nc.sync.dma_start(
            out=tw[:, :].rearrange("r (c k) -> r c k", k=K),
            in_=self.dixa[t].ap().rearrange("(c r) k -> r c k", r=16))
        nc.vector.tensor_copy(
            self.widx[0:16, t, :].rearrange("r (k c) -> r k c", c=8),
            tw[:, :].rearrange("r (c k) -> r k c", k=K))
        for lo, hi in ((16, 32), (32, 64), (64, 128)):
            nc.sync.dma_start(out=self.widx[lo:hi, t, :],
                              in_=self.widx[0:lo, t, :])

    def emit_meta(self, t):
        """Merged meta+feature row gather for tile t (both stages: 1792B rows)."""
        nc, pools = self.nc, self.pools
        gm = pools["gt"].tile([P, K, T0W], F32, tag="gt0")
        src = self.tba if self.st == 0 else self.gsrc
        nc.gpsimd.dma_gather(
            gm[:, :, :], src[:, :], self.widx[:, t, :],
            num_idxs=P * K, num_idxs_reg=P * K, elem_size=T0W)
        self.gms[t] = gm

    def emit_weights(self, t):
        """Direction-mask weights + dW diagonals for tile t."""
        nc, pools = self.nc, self.pools
        gm = self.gms[t]
        dirs = gm[:, :, 14:44].bitcast(BF16)   # [P,K,60] raw bf16
        perc = gm[:, :, 4:14].bitcast(BF16)    # [P,K,20]

        vec = pools["wk"].tile([P, K, 3], F32, tag="vec")
        nc.vector.tensor_tensor(
            out=vec[:, :, :], in0=gm[:, :, 0:3],
            in1=self.qxall[:, t, :].unsqueeze(1).to_broadcast([P, K, 3]),
            op=Sub)
        v2 = pools["wk"].tile([P, K, 3], F32, tag="v2")
        nc.vector.tensor_mul(v2[:, :, :], vec[:, :, :], vec[:, :, :])
        d2 = pools["wk"].tile([P, K], F32, tag="d2")
        nc.vector.reduce_sum(out=d2[:, :], in_=v2[:, :, :], axis=X)
        dist = pools["wk"].tile([P, K], F32, tag="dist")
        nc.scalar.activation(out=dist[:, :], in_=d2[:, :], func=Sqrt)
        # thr = GAMMA * (dist + eps)
        thr = pools["wk"].tile([P, K], F32, tag="thr")
        nc.vector.tensor_scalar(out=thr[:, :], in0=dist[:, :], scalar1=GAMMA,
                                scalar2=GAMMA * EPS_DIR, op0=Mult, op1=Add)

        dsq = pools["wk"].tile([P, K, M, 3], F32, tag="dsq")
        nc.vector.tensor_mul(
            dsq[:, :, :, :],
            dirs.rearrange("p k (m c) -> p k m c", c=3),
            dirs.rearrange("p k (m c) -> p k m c", c=3))
        nd2 = pools["wk"].tile([P, K, M], F32, tag="nd2")
        nc.vector.reduce_sum(out=nd2[:, :, :], in_=dsq[:, :, :, :], axis=X)
        nrm = pools["wk"].tile([P, K, M], F32, tag="nrm")
        nc.scalar.activation(out=nrm[:, :, :], in_=nd2[:, :, :], func=Sqrt)
        nc.vector.tensor_scalar_add(nrm[:, :, :], nrm[:, :, :], EPS_DIR)
        # thrm = thr * |d|  (mask test becomes |d.v| > thrm)
        thrm = pools["wk"].tile([P, K, M], F32, tag="thrm")
        nc.vector.tensor_tensor(
            out=thrm[:, :, :], in0=nrm[:, :, :],
            in1=thr[:, :].unsqueeze(2).to_broadcast([P, K, M]), op=Mult)

        prod = pools["wk"].tile([P, K, M, 3], F32, tag="prod")
        nc.vector.tensor_mul(
            prod[:, :, :, :],
            dirs.rearrange("p k (m c) -> p k m c", c=3),
            vec[:, :, :].unsqueeze(2).to_broadcast([P, K, M, 3]))
        simm = pools["wk"].tile([P, K, M], F32, tag="simm")
        nc.vector.reduce_sum(out=simm[:, :, :], in_=prod[:, :, :, :], axis=X)
        absm = pools["wk"].tile([P, K, M], F32, tag="absm")
        nc.scalar.activation(out=absm[:, :, :], in_=simm[:, :, :], func=Abs)
        mask = pools["wk"].tile([P, K, M], F32, tag="mask")
        nc.vector.tensor_tensor(out=mask[:, :, :], in0=absm[:, :, :],
                                in1=thrm[:, :, :], op=IsGt)
        mw = pools["wk"].tile([P, K, M], F32, tag="mw")
        nc.vector.tensor_mul(mw[:, :, :], mask[:, :, :], perc)
        dkw = pools["wk"].tile([P, K], F32, tag="dkw")
        nc.vector.reduce_sum(out=dkw[:, :], in_=mw[:, :, :], axis=X)

        dkws = pools["wk"].tile([P, 1], F32, tag="dkws")
        nc.vector.reduce_sum(out=dkws[:, :], in_=dkw[:, :], axis=X)
        nc.vector.tensor_scalar_add(dkws[:, :], dkws[:, :], 1e-8)
        r1 = pools["wk"].tile([P, 1], F32, tag="r1")
        nc.vector.reciprocal(r1[:, :], dkws[:, :])
        wn = pools["wk"].tile([P, K], F32, tag="wn")
        nc.vector.tensor_tensor(out=wn[:, :], in0=dkw[:, :],
                                in1=r1[:, 0:1].to_broadcast([P, K]), op=Mult)
        nc.vector.tensor_scalar_add(wn[:, :], wn[:, :], 1e-6 + 1e-10)
        nr2 = pools["wk"].tile([P, 1], F32, tag="nr2")
        nc.vector.reduce_sum(out=nr2[:, :], in_=wn[:, :], axis=X)
        nc.vector.tensor_scalar_add(nr2[:, :], nr2[:, :], 1e-8)
        r2 = pools["wk"].tile([P, 1], F32, tag="r2")
        nc.vector.reciprocal(r2[:, :], nr2[:, :])
        rr = pools["wk"].tile([P, 1], F32, tag="rr")
        nc.vector.tensor_mul(rr[:, :], r2[:, :], dkws[:, :])
        wp = pools["wk"].tile([P, K], F32, tag="wp")
        nc.vector.tensor_tensor(out=wp[:, :], in0=wn[:, :],
                                in1=rr[:, 0:1].to_broadcast([P, K]), op=Mult)

        nc.vector.tensor_copy(self.wpall[:, t, :], wp[:, :])

        par = pools["wk"].tile([P, 1], F32, tag="par")
        nc.gpsimd.partition_all_reduce(par[:, :], dkws[:, :], channels=P,
                                       reduce_op=bass_isa.ReduceOp.add)
        nc.vector.tensor_add(self.acc[:, :], self.acc[:, :], par[:, :])

    def emit_interp(self, t):
        """PE interpolation from the merged gather's feature region."""
        nc, pools = self.nc, self.pools
        gf = self.gms[t][:, :, 64:T0W].bitcast(BF16)   # [P,K,768]
        dW = pools["dw"].tile([P, K, P], BF16, tag="dW")
        for k in range(K):
            nc.scalar.activation(out=dW[:, k, :], in_=self.identF[:, :],
                                 func=Copy, scale=self.wpall[:, t, k:k + 1])
        po = pools["po"].tile([P, D], F32, tag="po")
        for k in range(K):
            for c0, c1 in ((0, 512), (512, D)):
                nc.tensor.matmul(out=po[:, c0:c1], lhsT=dW[:, k, :],
                                 rhs=gf[:, k, c0:c1],
                                 start=(k == 0), stop=(k == K - 1))
        # stage the result out of PSUM so later finalize (gated on the
        # AllReduce scalar) never backs up the interp pipeline
        nc.scalar.activation(out=self.f1keep[:, t, :], in_=po[:, :], func=Copy)

    def emit_allreduce(self):
        nc, pools = self.nc, self.pools
        nc.sync.dma_start(out=self.sum_in.ap()[:, :], in_=self.acc[0:1, 0:1])
        nc.gpsimd.collective_compute(
            "AllReduce", mybir.AluOpType.add, replica_groups=RG,
            ins=[self.sum_in.ap()], outs=[self.sum_out.ap()],
        )
        sg = pools["tbl"].tile([P, 1], F32, tag=f"sg_{self.st}")
        nc.sync.dma_start(out=sg[0:1, :], in_=self.sum_out.ap()[:, :])
        sgb = pools["tbl"].tile([P, 1], F32, tag=f"sgb_{self.st}")
        nc.gpsimd.partition_broadcast(sgb[:, :], sg[0:1, :], channels=P)
        scal = pools["tbl"].tile([P, 1], F32, tag=f"scal_{self.st}")
        nc.vector.tensor_scalar(out=scal[:, :], in0=sgb[:, :],
                                scalar1=C_SCAL / self.NT, scalar2=1e-8,
                                op0=Mult, op1=Add)
        self.scal = scal

    def emit_finalize(self, t):
        """res = normalize(f1 + scal * p1) -> out rows (bf16-packed)."""
        nc, pools = self.nc, self.pools
        rs = slice(t * P, (t + 1) * P)
        p1t = pools["fw"].tile([P, D], BF16, tag="p1t")
        nc.sync.dma_start(out=p1t[:, :], in_=self.p1a[rs, :].bitcast(BF16))
        f2 = pools["fw"].tile([P, D], F32, tag="f2")
        nc.scalar.activation(out=f2[:, :], in_=p1t[:, :], func=Copy,
                             scale=self.scal[:, 0:1])
        o = pools["fw"].tile([P, D], F32, tag="o")
        nc.vector.tensor_add(o[:, :], self.f1keep[:, t, :], f2[:, :])
        junk = pools["fw"].tile([P, D], BF16, tag="res")
        ss = pools["wk"].tile([P, 1], F32, tag="ss")
        nc.scalar.activation(out=junk[:, :], in_=o[:, :], func=Square,
                             accum_out=ss[:, :])
        nn = pools["wk"].tile([P, 1], F32, tag="nn")
        nc.scalar.activation(out=nn[:, :], in_=ss[:, :], func=Sqrt)
        nc.vector.tensor_scalar_max(nn[:, :], nn[:, :], 1e-12)
        res = pools["fw"].tile([P, D], BF16, tag="res")
        nc.gpsimd.normalize_recip(res[:, :], o[:, :], nn[:, :])
        if self.st == 0:
            dst = self.ora[rs, T1W:T0W].bitcast(BF16)  # p2sx feature region
        else:
            dst = self.ora[rs, :].bitcast(BF16)  # out1: f32 words of bf16 pairs
        nc.sync.dma_start(out=dst, in_=res[:, :])


def build():
    if "nc" in _CACHE:
        return _CACHE["nc"]
    nc = bacc.Bacc("TRN2", num_devices=NCORES)

    t0 = nc.dram_tensor("t0", [ST0["S"], T0W], F32, kind="ExternalInput")
    t1m = nc.dram_tensor("t1m", [ST1["S"], T1W], F32, kind="ExternalInput")
    t1own = nc.dram_tensor("t1own", [ST0["Q"], T1W], F32, kind="ExternalInput")
    q0 = nc.dram_tensor("q0", [ST0["Q"], 3], F32, kind="ExternalInput")
    q1 = nc.dram_tensor("q1", [ST1["Q"], 3], F32, kind="ExternalInput")
    p10 = nc.dram_tensor("p10", [ST0["Q"], D // 2], F32, kind="ExternalInput")
    p11 = nc.dram_tensor("p11", [ST1["Q"], D // 2], F32, kind="ExternalInput")

    out1 = nc.dram_tensor("out1", [ST1["Q"], D // 2], F32, kind="ExternalOutput")

    dix0 = [nc.dram_tensor(f"dix0_{t}", [P, K], I16)
            for t in range(ST0["Q"] // P)]
    dix1 = [nc.dram_tensor(f"dix1_{t}", [P, K], I16)
            for t in range(ST1["Q"] // P)]
    # stage-0 output as full merged rows [own meta | own features]; the
    # AllGather of these rows forms the complete stage-1 gather table
    p2sx = nc.dram_tensor("p2sx", [ST0["Q"], T0W], F32)
    tm1x = nc.dram_tensor("tm1x", [ST1["S"], T0W], F32, addr_space="Shared")
    s0in = nc.dram_tensor("s0in", [1, 1], F32)
    s0out = nc.dram_tensor("s0out", [1, 1], F32, addr_space="Shared")
    s1in = nc.dram_tensor("s1in", [1, 1], F32)
    s1out = nc.dram_tensor("s1out", [1, 1], F32, addr_space="Shared")

    with TileContext(nc) as tc:
        import contextlib
        with contextlib.ExitStack() as ctx:
            pools = {
                "const": ctx.enter_context(tc.tile_pool(name="const", bufs=1)),
                "tbl": ctx.enter_context(tc.tile_pool(name="tbl", bufs=1)),
                "mw": ctx.enter_context(tc.tile_pool(name="mw", bufs=2)),
                "wk": ctx.enter_context(tc.tile_pool(name="wk", bufs=2)),
                "neg": ctx.enter_context(tc.tile_pool(name="neg", bufs=2)),
                "ix": ctx.enter_context(tc.tile_pool(name="ix", bufs=4)),
                "gt": ctx.enter_context(tc.tile_pool(name="gt", bufs=3)),
                "dw": ctx.enter_context(tc.tile_pool(name="dw", bufs=2)),
                "fw": ctx.enter_context(tc.tile_pool(name="fw", bufs=3)),
                "pt": ctx.enter_context(tc.tile_pool(name="pt", bufs=1, space="PSUM")),
                "pe": ctx.enter_context(tc.tile_pool(name="pe", bufs=3, space="PSUM")),
                "po": ctx.enter_context(tc.tile_pool(name="po", bufs=2, space="PSUM")),
            }
            identF = pools["const"].tile([P, P], F32, tag="identF")
            make_identity(nc, identF[:, :])
            identB = pools["const"].tile([P, P], BF16, tag="identB")
            nc.scalar.activation(out=identB[:, :], in_=identF[:, :], func=Copy)

            s0 = Stage(nc, pools, identF, identB, st=0, S=ST0["S"], Q=ST0["Q"],
                       NT=ST0["NT"], tbl=t0, tw=None, qxyz=q0, p1=p10,
                       out_rows=p2sx, sum_in=s0in, sum_out=s0out)
            s1 = Stage(nc, pools, identF, identB, st=1, S=ST1["S"], Q=ST1["Q"],
                       NT=ST1["NT"], tbl=t1m, tw=None, qxyz=q1, p1=p11,
                       out_rows=out1, sum_in=s1in, sum_out=s1out)
            s0.wpall = pools["tbl"].tile([P, s0.n_qt, K], F32, tag="wp0")
            s1.wpall = pools["tbl"].tile([P, s1.n_qt, K], F32, tag="wp1")
            s0.dixa = dix0
            s1.dixa = dix1
            s1.gsrc = tm1x.ap()
            # fill this core's merged-row meta region early
            nc.sync.dma_start(out=p2sx.ap()[:, 0:T1W], in_=t1own.ap()[:, :])

            s0.f1keep = pools["tbl"].tile([P, s0.n_qt, D], F32, tag="f1k0")
            s1.f1keep = pools["tbl"].tile([P, s1.n_qt, D], BF16, tag="f1k1")

            # stage-0 chain (gates the collectives); gpsimd queue order is
            # [s0 gathers, AR0, AllGather, s1 metas, s1 feats, AR1] so no
            # gather ever sits behind a blocking collective it doesn't need.
            s0.emit_tables()
            s1.emit_tables()
            for t in range(s0.n_qt):
                s0.emit_rank(t)
            # early stage-1 rank tiles fill DVE while s0 gathers run
            for t in range(6):
                s1.emit_rank(t)
            for t in range(s0.n_qt):
                s0.emit_meta(t)
                s0.emit_weights(t)
                s0.emit_interp(t)
            s0.emit_allreduce()
            # more rank tiles cover the AllReduce latency ahead of the
            # scal0-gated s0 finalize in the DVE queue
            for t in range(6, 11):
                s1.emit_rank(t)
            for t in range(s0.n_qt):
                s0.emit_finalize(t)
            for t in range(11, s1.n_qt):
                s1.emit_rank(t)
            nc.gpsimd.collective_compute(
                "AllGather", mybir.AluOpType.bypass, replica_groups=RG,
                ins=[p2sx.ap()], outs=[tm1x.ap()],
            )
            for t in range(s1.n_qt):
                s1.emit_meta(t)
                s1.emit_weights(t)
            for t in range(s1.n_qt):
                s1.emit_interp(t)
            s1.emit_allreduce()
            for t in range(s1.n_qt):
                s1.emit_finalize(t)

    nc.compile()
    _CACHE["nc"] = nc
    return nc


def _pack_bf16(x):
    """[r, n] float -> [r, n//2] float32 words holding bf16 pairs."""
    b = np.ascontiguousarray(x, dtype=np.float32).astype(ml_dtypes.bfloat16)
    return b.view(np.uint16).reshape(x.shape[0], -1).view(np.uint32).view(np.float32)


def _pack(inputs):
    xyz_c = np.ascontiguousarray(inputs["xyz_c"][0], dtype=np.float32)
    xyz_m = np.ascontiguousarray(inputs["xyz_m"][0], dtype=np.float32)
    xyz_f = np.ascontiguousarray(inputs["xyz_f"][0], dtype=np.float32)
    x_c = np.ascontiguousarray(inputs["x_c"][0], dtype=np.float32)
    x_m = np.ascontiguousarray(inputs["x_m"][0], dtype=np.float32)
    x_f = np.ascontiguousarray(inputs["x_f"][0], dtype=np.float32)
    perc_c = np.ascontiguousarray(inputs["perc_c"][0], dtype=np.float32)
    dir_c = np.ascontiguousarray(inputs["dir_c"][0], dtype=np.float32)
    perc_m = np.ascontiguousarray(inputs["perc_m"][0], dtype=np.float32)
    dir_m = np.ascontiguousarray(inputs["dir_m"][0], dtype=np.float32)

    t0 = np.zeros((ST0["S"], T0W), np.float32)
    t0[:, 0:3] = xyz_c
    t0[:, 4:14] = _pack_bf16(perc_c)
    t0[:, 14:44] = _pack_bf16(dir_c.reshape(ST0["S"], 60))
    t0[:, 64:T0W] = _pack_bf16(x_c)

    t1m = np.zeros((ST1["S"], T1W), np.float32)
    t1m[:, 0:3] = xyz_m
    t1m[:, 4:14] = _pack_bf16(perc_m)
    t1m[:, 14:44] = _pack_bf16(dir_m.reshape(ST1["S"], 60))

    in_maps = []
    for c in range(NCORES):
        r0 = slice(c * ST0["Q"], (c + 1) * ST0["Q"])
        r1 = slice(c * ST1["Q"], (c + 1) * ST1["Q"])
        in_maps.append({
            "t0": t0,
            "t1m": t1m,
            "t1own": np.ascontiguousarray(t1m[r0]),
            "q0": np.ascontiguousarray(xyz_m[r0]),
            "q1": np.ascontiguousarray(xyz_f[r1]),
            "p10": _pack_bf16(x_m[r0]),
            "p11": _pack_bf16(x_f[r1]),
        })
    return in_maps


def run_sharded(inputs, trace=False, tmpdir=None):
    """Build + run; returns (full_output, BassKernelResults)."""
    from concourse.bass_utils import run_bass_kernel_spmd
    nc = build()
    in_maps = _pack(inputs)
    res = run_bass_kernel_spmd(nc, in_maps, list(range(NCORES)), trace=trace,
                               tmpdir=tmpdir)
    outs = []
    for c in range(NCORES):
        o = res.results[c]["out1"]  # [Q1, 384] f32 words of bf16 pairs
        outs.append(np.ascontiguousarray(o).view(ml_dtypes.bfloat16)
                    .astype(np.float32).reshape(ST1["Q"], D))
    out = np.concatenate(outs, axis=0)
    return out.reshape(1, ST1["NT"], D), res


def kernel(**inputs) -> np.ndarray:
    out, _ = run_sharded(inputs, trace=False)
    return out


# revision 79
# speedup vs baseline: 1.0419x; 1.0419x over previous
"""Trainium2 Bass kernel for nn_DecNP (two-stage KNN feature propagation).

Algorithm (per stage): rank coarse points per query with a PE matmul
(coords split bf16 hi/lo so a contraction-16 bf16 matmul reproduces the
fp32 ranking to ~2^-17), top-8 via DVE max8/find_index8, one batched
indirect DMA per query tile to gather neighbour meta rows (xyz fp32,
percentages/directions bf16), direction-mask weights on DVE with the
unnormalized-direction compare |d.v| > g*(dist+eps)*|d|, feature gather
(stage 0: merged row, stage 1: straight from the AllGather output), PE
diagonal-weight interpolation, fused residual + L2 normalize.

Sharding: query rows split across 8 cores.  Stage-0 output is
AllGather'd (it is the feature table of stage 1); the scalar mean of
de_k_weight_sum is AllReduce'd per stage, launched as soon as the last
weight tile finishes so the finalize never waits on it.
"""
import sys

for _p in ("/opt/trn_rl_repo", "/root/.axon_site/_ro/trn_rl_repo", "/root/.axon_site"):
    if _p not in sys.path:
        sys.path.append(_p)

import numpy as np
import ml_dtypes

import concourse.bacc as bacc
import concourse.bass as bass
import concourse.bass_isa as bass_isa
import concourse.mybir as mybir
from concourse.masks import make_identity
from concourse.tile import TileContext

NCORES = 8
P = 128
D = 768
K = 8
M = 20
GAMMA = 0.85
EPS_DIR = 1e-8
BF16 = mybir.dt.bfloat16
F32 = mybir.dt.float32
U32 = mybir.dt.uint32
U16 = mybir.dt.uint16
I16 = mybir.dt.int16
X = mybir.AxisListType.X
Copy = mybir.ActivationFunctionType.Copy
Sqrt = mybir.ActivationFunctionType.Sqrt
Square = mybir.ActivationFunctionType.Square
Abs = mybir.ActivationFunctionType.Abs
Mult = mybir.AluOpType.mult
Add = mybir.AluOpType.add
Sub = mybir.AluOpType.subtract
IsGt = mybir.AluOpType.is_gt

# table row layouts, in fp32 words
# t0 (stage 0, merged): 0:3 xyz | 4:14 perc bf16 | 14:44 dirs bf16 | 64:448 feat bf16
# t1m (stage 1, meta):  0:3 xyz | 4:14 perc bf16 | 14:44 dirs bf16 | pad to 64
T0W = 448
T1W = 64

ST0 = dict(S=1024, Q=512, NT=4096)
ST1 = dict(S=4096, Q=2048, NT=16384)
C_SCAL = 0.3  # N == 4*S in both stages

RG = [list(range(NCORES))]

_CACHE = {}


class Stage:
    def __init__(self, nc, pools, identF, identB, *, st, S, Q, NT, tbl, tw,
                 qxyz, p1, out_rows, sum_in, sum_out):
        self.__dict__.update(locals())
        self.n_st = S // P
        self.n_qt = Q // P
        self.tba = tbl.ap()
        self.p1a = p1.ap()
        self.ora = out_rows.ap()

    def emit_tables(self):
        """c8 = [chi;clo;chi;clo] rows (x,y,z,|s|^2 hi/lo bf16), q8 likewise."""
        nc, pools = self.nc, self.pools
        st, S, Q = self.st, self.S, self.Q
        c8 = pools["tbl"].tile([16, S], BF16, tag=f"c8_{st}")
        self.c8 = c8
        for i in range(self.n_st):
            rs = slice(i * P, (i + 1) * P)
            xyz = pools["mw"].tile([P, 3], F32, tag="xyz")
            nc.sync.dma_start(out=xyz[:, :], in_=self.tba[rs, 0:3])
            ca = pools["mw"].tile([P, 4], F32, tag="ca")
            nc.vector.tensor_copy(ca[:, 0:3], xyz[:, :])
            sq3 = pools["mw"].tile([P, 3], F32, tag="sq3")
            nc.vector.tensor_mul(sq3[:, :], xyz[:, :], xyz[:, :])
            nc.vector.reduce_sum(out=ca[:, 3:4], in_=sq3[:, :], axis=X)
            # hl = [chi | clo | chi | clo] in the free dim, then one transpose
            hl = pools["mw"].tile([P, 16], BF16, tag="hl")
            nc.scalar.activation(out=hl[:, 0:4], in_=ca[:, :], func=Copy)
            nc.vector.tensor_tensor(out=hl[:, 4:8], in0=ca[:, :],
                                    in1=hl[:, 0:4], op=Sub)
            nc.vector.tensor_copy(hl[:, 8:16], hl[:, 0:8])
            ptp = pools["pt"].tile([16, P], BF16, tag="ptp")
            nc.tensor.transpose(out=ptp[:, :], in_=hl[:, :],
                                identity=self.identB[:, :])
            nc.scalar.activation(out=c8[:, rs], in_=ptp[:, :], func=Copy)

        qxall = pools["tbl"].tile([P, self.n_qt, 3], F32, tag=f"qxall_{st}")
        self.qxall = qxall
        q8 = pools["tbl"].tile([16, Q], BF16, tag=f"q8_{st}")
        self.q8 = q8
        qxa = self.qxyz.ap()
        for t in range(self.n_qt):
            rs = slice(t * P, (t + 1) * P)
            nc.sync.dma_start(out=qxall[:, t, :], in_=qxa[rs, :])
            qt4 = pools["mw"].tile([P, 4], F32, tag="qt4")
            nc.vector.tensor_scalar_mul(qt4[:, 0:3], qxall[:, t, :], 2.0)
            nc.vector.memset(qt4[:, 3:4], -1.0)
            # hl = [qhi | qhi | qlo | qlo] in the free dim, then one transpose
            hl = pools["mw"].tile([P, 16], BF16, tag="hl")
            nc.scalar.activation(out=hl[:, 0:4], in_=qt4[:, :], func=Copy)
            nc.vector.tensor_copy(hl[:, 4:8], hl[:, 0:4])
            nc.vector.tensor_tensor(out=hl[:, 8:12], in0=qt4[:, :],
                                    in1=hl[:, 0:4], op=Sub)
            nc.vector.tensor_copy(hl[:, 12:16], hl[:, 8:12])
            ptp = pools["pt"].tile([16, P], BF16, tag="ptp")
            nc.tensor.transpose(out=ptp[:, :], in_=hl[:, :],
                                identity=self.identB[:, :])
            nc.scalar.activation(out=q8[:, rs], in_=ptp[:, :], func=Copy)

        # wrapped dma_gather index tiles: [16 partitions x 64] per query tile,
        # replicated to all 8 gpsimd-core stripes (128 partitions total);
        # one resident tile per query tile so no false WAR coupling arises
        self.widxs = [pools["tbl"].tile([P, 64], I16, tag=f"widx_{st}_{t}",
                                        name=f"widx_{st}_{t}")
                      for t in range(self.n_qt)]
        self.acc = pools["tbl"].tile([P, 1], F32, tag=f"acc_{st}")
        nc.vector.memset(self.acc[:, :], 0.0)
        self.gms = {}

    def emit_rank(self, t):
        """negE = 2q.c - |c|^2 via bf16 hi/lo matmul; top-8 indices."""
        nc, pools = self.nc, self.pools
        S = self.S
        rs = slice(t * P, (t + 1) * P)
        negE = pools["neg"].tile([P, S], F32, tag="negE")
        for c in range(S // 512):
            pe = pools["pe"].tile([P, 512], F32, tag="pe")
            nc.tensor.matmul(out=pe[:, :], lhsT=self.q8[:, rs],
                             rhs=self.c8[:, c * 512:(c + 1) * 512],
                             start=True, stop=True)
            nc.scalar.activation(out=negE[:, c * 512:(c + 1) * 512],
                                 in_=pe[:, :], func=Copy)
        best = pools["wk"].tile([P, 8], F32, tag="best")
        ix = pools["ix"].tile([P, K], U16, tag="ix")
        nc.vector.max(out=best[:, :], in_=negE[:, :])
        nc.vector.max_index(out=ix[:, :], in_max=best[:, :],
                            in_values=negE[:, :])
        nc.sync.dma_start(out=self.dixa[t].ap(), in_=ix[:, :].bitcast(I16))
        # build the wrapped dma_gather index layout, widx[r, 8k+c] =
        # idx[16c+r, k], via a DRAM bounce (contiguous 16B runs) + an
        # on-chip free-dim transpose + log-doubling partition replication
        tw = pools["ix"].tile([16, 64], I16, tag="tw")
        nc.sync.dma_start(
            out=tw[:, :].rearrange("r (c k) -> r c k", k=K),
            in_=self.dixa[t].ap().rearrange("(c r) k -> r c k", r=16))
        wx = self.widxs[t]
        nc.vector.tensor_copy(
            wx[0:16, :].rearrange("r (k c) -> r k c", c=8),
            tw[:, :].rearrange("r (c k) -> r k c", k=K))
        for lo, hi in ((16, 32), (32, 64), (64, 128)):
            nc.sync.dma_start(out=wx[lo:hi, :], in_=wx[0:lo, :])

    def emit_meta(self, t):
        """Merged meta+feature row gather for tile t (both stages: 1792B rows)."""
        nc, pools = self.nc, self.pools
        gm = pools["gt"].tile([P, K, T0W], F32, tag="gt0")
        src = self.tba if self.st == 0 else self.gsrc
        nc.gpsimd.dma_gather(
            gm[:, :, :], src[:, :], self.widxs[t][:, :],
            num_idxs=P * K, num_idxs_reg=P * K, elem_size=T0W)
        self.gms[t] = gm

    def emit_weights(self, t):
        """Direction-mask weights + dW diagonals for tile t."""
        nc, pools = self.nc, self.pools
        gm = self.gms[t]
        dirs = gm[:, :, 14:44].bitcast(BF16)   # [P,K,60] raw bf16
        perc = gm[:, :, 4:14].bitcast(BF16)    # [P,K,20]

        vec = pools["wk"].tile([P, K, 3], F32, tag="vec")
        nc.vector.tensor_tensor(
            out=vec[:, :, :], in0=gm[:, :, 0:3],
            in1=self.qxall[:, t, :].unsqueeze(1).to_broadcast([P, K, 3]),
            op=Sub)
        v2 = pools["wk"].tile([P, K, 3], F32, tag="v2")
        nc.vector.tensor_mul(v2[:, :, :], vec[:, :, :], vec[:, :, :])
        d2 = pools["wk"].tile([P, K], F32, tag="d2")
        nc.vector.reduce_sum(out=d2[:, :], in_=v2[:, :, :], axis=X)
        dist = pools["wk"].tile([P, K], F32, tag="dist")
        nc.scalar.activation(out=dist[:, :], in_=d2[:, :], func=Sqrt)
        # thr = GAMMA * (dist + eps)
        thr = pools["wk"].tile([P, K], F32, tag="thr")
        nc.vector.tensor_scalar(out=thr[:, :], in0=dist[:, :], scalar1=GAMMA,
                                scalar2=GAMMA * EPS_DIR, op0=Mult, op1=Add)

        dsq = pools["wk"].tile([P, K, M, 3], F32, tag="dsq")
        nc.vector.tensor_mul(
            dsq[:, :, :, :],
            dirs.rearrange("p k (m c) -> p k m c", c=3),
            dirs.rearrange("p k (m c) -> p k m c", c=3))
        nd2 = pools["wk"].tile([P, K, M], F32, tag="nd2")
        nc.vector.reduce_sum(out=nd2[:, :, :], in_=dsq[:, :, :, :], axis=X)
        nrm = pools["wk"].tile([P, K, M], F32, tag="nrm")
        nc.scalar.activation(out=nrm[:, :, :], in_=nd2[:, :, :], func=Sqrt)
        nc.vector.tensor_scalar_add(nrm[:, :, :], nrm[:, :, :], EPS_DIR)
        # thrm = thr * |d|  (mask test becomes |d.v| > thrm)
        thrm = pools["wk"].tile([P, K, M], F32, tag="thrm")
        nc.vector.tensor_tensor(
            out=thrm[:, :, :], in0=nrm[:, :, :],
            in1=thr[:, :].unsqueeze(2).to_broadcast([P, K, M]), op=Mult)

        prod = pools["wk"].tile([P, K, M, 3], F32, tag="prod")
        nc.vector.tensor_mul(
            prod[:, :, :, :],
            dirs.rearrange("p k (m c) -> p k m c", c=3),
            vec[:, :, :].unsqueeze(2).to_broadcast([P, K, M, 3]))
        simm = pools["wk"].tile([P, K, M], F32, tag="simm")
        nc.vector.reduce_sum(out=simm[:, :, :], in_=prod[:, :, :, :], axis=X)
        absm = pools["wk"].tile([P, K, M], F32, tag="absm")
        nc.scalar.activation(out=absm[:, :, :], in_=simm[:, :, :], func=Abs)
        mask = pools["wk"].tile([P, K, M], F32, tag="mask")
        nc.vector.tensor_tensor(out=mask[:, :, :], in0=absm[:, :, :],
                                in1=thrm[:, :, :], op=IsGt)
        mw = pools["wk"].tile([P, K, M], F32, tag="mw")
        nc.vector.tensor_mul(mw[:, :, :], mask[:, :, :], perc)
        dkw = pools["wk"].tile([P, K], F32, tag="dkw")
        nc.vector.reduce_sum(out=dkw[:, :], in_=mw[:, :, :], axis=X)

        dkws = pools["wk"].tile([P, 1], F32, tag="dkws")
        nc.vector.reduce_sum(out=dkws[:, :], in_=dkw[:, :], axis=X)
        nc.vector.tensor_scalar_add(dkws[:, :], dkws[:, :], 1e-8)
        r1 = pools["wk"].tile([P, 1], F32, tag="r1")
        nc.vector.reciprocal(r1[:, :], dkws[:, :])
        wn = pools["wk"].tile([P, K], F32, tag="wn")
        nc.vector.tensor_tensor(out=wn[:, :], in0=dkw[:, :],
                                in1=r1[:, 0:1].to_broadcast([P, K]), op=Mult)
        nc.vector.tensor_scalar_add(wn[:, :], wn[:, :], 1e-6 + 1e-10)
        nr2 = pools["wk"].tile([P, 1], F32, tag="nr2")
        nc.vector.reduce_sum(out=nr2[:, :], in_=wn[:, :], axis=X)
        nc.vector.tensor_scalar_add(nr2[:, :], nr2[:, :], 1e-8)
        r2 = pools["wk"].tile([P, 1], F32, tag="r2")
        nc.vector.reciprocal(r2[:, :], nr2[:, :])
        rr = pools["wk"].tile([P, 1], F32, tag="rr")
        nc.vector.tensor_mul(rr[:, :], r2[:, :], dkws[:, :])
        wp = pools["wk"].tile([P, K], F32, tag="wp")
        nc.vector.tensor_tensor(out=wp[:, :], in0=wn[:, :],
                                in1=rr[:, 0:1].to_broadcast([P, K]), op=Mult)

        nc.vector.tensor_copy(self.wpall[:, t, :], wp[:, :])

        par = pools["wk"].tile([P, 1], F32, tag="par")
        nc.gpsimd.partition_all_reduce(par[:, :], dkws[:, :], channels=P,
                                       reduce_op=bass_isa.ReduceOp.add)
        nc.vector.tensor_add(self.acc[:, :], self.acc[:, :], par[:, :])

    def emit_interp(self, t):
        """PE interpolation from the merged gather's feature region."""
        nc, pools = self.nc, self.pools
        gf = self.gms[t][:, :, 64:T0W].bitcast(BF16)   # [P,K,768]
        dW = pools["dw"].tile([P, K, P], BF16, tag="dW")
        for k in range(K):
            nc.scalar.activation(out=dW[:, k, :], in_=self.identF[:, :],
                                 func=Copy, scale=self.wpall[:, t, k:k + 1])
        po = pools["po"].tile([P, D], F32, tag="po")
        for k in range(K):
            for c0, c1 in ((0, 512), (512, D)):
                nc.tensor.matmul(out=po[:, c0:c1], lhsT=dW[:, k, :],
                                 rhs=gf[:, k, c0:c1],
                                 start=(k == 0), stop=(k == K - 1))
        # stage the result out of PSUM so later finalize (gated on the
        # AllReduce scalar) never backs up the interp pipeline
        nc.scalar.activation(out=self.f1keep[:, t, :], in_=po[:, :], func=Copy)

    def emit_allreduce(self):
        nc, pools = self.nc, self.pools
        nc.sync.dma_start(out=self.sum_in.ap()[:, :], in_=self.acc[0:1, 0:1])
        nc.gpsimd.collective_compute(
            "AllReduce", mybir.AluOpType.add, replica_groups=RG,
            ins=[self.sum_in.ap()], outs=[self.sum_out.ap()],
        )
        sg = pools["tbl"].tile([P, 1], F32, tag=f"sg_{self.st}")
        nc.sync.dma_start(out=sg[0:1, :], in_=self.sum_out.ap()[:, :])
        sgb = pools["tbl"].tile([P, 1], F32, tag=f"sgb_{self.st}")
        nc.gpsimd.partition_broadcast(sgb[:, :], sg[0:1, :], channels=P)
        scal = pools["tbl"].tile([P, 1], F32, tag=f"scal_{self.st}")
        nc.vector.tensor_scalar(out=scal[:, :], in0=sgb[:, :],
                                scalar1=C_SCAL / self.NT, scalar2=1e-8,
                                op0=Mult, op1=Add)
        self.scal = scal

    def emit_finalize(self, t):
        """res = normalize(f1 + scal * p1) -> out rows (bf16-packed)."""
        nc, pools = self.nc, self.pools
        rs = slice(t * P, (t + 1) * P)
        p1t = pools["fw"].tile([P, D], BF16, tag="p1t")
        nc.sync.dma_start(out=p1t[:, :], in_=self.p1a[rs, :].bitcast(BF16))
        f2 = pools["fw"].tile([P, D], F32, tag="f2")
        nc.scalar.activation(out=f2[:, :], in_=p1t[:, :], func=Copy,
                             scale=self.scal[:, 0:1])
        o = pools["fw"].tile([P, D], F32, tag="o")
        nc.vector.tensor_add(o[:, :], self.f1keep[:, t, :], f2[:, :])
        junk = pools["fw"].tile([P, D], BF16, tag="res")
        ss = pools["wk"].tile([P, 1], F32, tag="ss")
        nc.scalar.activation(out=junk[:, :], in_=o[:, :], func=Square,
                             accum_out=ss[:, :])
        nn = pools["wk"].tile([P, 1], F32, tag="nn")
        nc.scalar.activation(out=nn[:, :], in_=ss[:, :], func=Sqrt)
        nc.vector.tensor_scalar_max(nn[:, :], nn[:, :], 1e-12)
        res = pools["fw"].tile([P, D], BF16, tag="res")
        nc.gpsimd.normalize_recip(res[:, :], o[:, :], nn[:, :])
        if self.st == 0:
            dst = self.ora[rs, T1W:T0W].bitcast(BF16)  # p2sx feature region
        else:
            dst = self.ora[rs, :].bitcast(BF16)  # out1: f32 words of bf16 pairs
        nc.sync.dma_start(out=dst, in_=res[:, :])


def build():
    if "nc" in _CACHE:
        return _CACHE["nc"]
    nc = bacc.Bacc("TRN2", num_devices=NCORES)

    t0 = nc.dram_tensor("t0", [ST0["S"], T0W], F32, kind="ExternalInput")
    t1m = nc.dram_tensor("t1m", [ST1["S"], T1W], F32, kind="ExternalInput")
    t1own = nc.dram_tensor("t1own", [ST0["Q"], T1W], F32, kind="ExternalInput")
    q0 = nc.dram_tensor("q0", [ST0["Q"], 3], F32, kind="ExternalInput")
    q1 = nc.dram_tensor("q1", [ST1["Q"], 3], F32, kind="ExternalInput")
    p10 = nc.dram_tensor("p10", [ST0["Q"], D // 2], F32, kind="ExternalInput")
    p11 = nc.dram_tensor("p11", [ST1["Q"], D // 2], F32, kind="ExternalInput")

    out1 = nc.dram_tensor("out1", [ST1["Q"], D // 2], F32, kind="ExternalOutput")

    dix0 = [nc.dram_tensor(f"dix0_{t}", [P, K], I16)
            for t in range(ST0["Q"] // P)]
    dix1 = [nc.dram_tensor(f"dix1_{t}", [P, K], I16)
            for t in range(ST1["Q"] // P)]
    # stage-0 output as full merged rows [own meta | own features]; the
    # AllGather of these rows forms the complete stage-1 gather table
    p2sx = nc.dram_tensor("p2sx", [ST0["Q"], T0W], F32)
    tm1x = nc.dram_tensor("tm1x", [ST1["S"], T0W], F32, addr_space="Shared")
    s0in = nc.dram_tensor("s0in", [1, 1], F32)
    s0out = nc.dram_tensor("s0out", [1, 1], F32, addr_space="Shared")
    s1in = nc.dram_tensor("s1in", [1, 1], F32)
    s1out = nc.dram_tensor("s1out", [1, 1], F32, addr_space="Shared")

    with TileContext(nc) as tc:
        import contextlib
        with contextlib.ExitStack() as ctx:
            pools = {
                "const": ctx.enter_context(tc.tile_pool(name="const", bufs=1)),
                "tbl": ctx.enter_context(tc.tile_pool(name="tbl", bufs=1)),
                "mw": ctx.enter_context(tc.tile_pool(name="mw", bufs=2)),
                "wk": ctx.enter_context(tc.tile_pool(name="wk", bufs=2)),
                "neg": ctx.enter_context(tc.tile_pool(name="neg", bufs=2)),
                "ix": ctx.enter_context(tc.tile_pool(name="ix", bufs=4)),
                "gt": ctx.enter_context(tc.tile_pool(name="gt", bufs=3)),
                "dw": ctx.enter_context(tc.tile_pool(name="dw", bufs=2)),
                "fw": ctx.enter_context(tc.tile_pool(name="fw", bufs=3)),
                "pt": ctx.enter_context(tc.tile_pool(name="pt", bufs=1, space="PSUM")),
                "pe": ctx.enter_context(tc.tile_pool(name="pe", bufs=3, space="PSUM")),
                "po": ctx.enter_context(tc.tile_pool(name="po", bufs=2, space="PSUM")),
            }
            identF = pools["const"].tile([P, P], F32, tag="identF")
            make_identity(nc, identF[:, :])
            identB = pools["const"].tile([P, P], BF16, tag="identB")
            nc.scalar.activation(out=identB[:, :], in_=identF[:, :], func=Copy)

            s0 = Stage(nc, pools, identF, identB, st=0, S=ST0["S"], Q=ST0["Q"],
                       NT=ST0["NT"], tbl=t0, tw=None, qxyz=q0, p1=p10,
                       out_rows=p2sx, sum_in=s0in, sum_out=s0out)
            s1 = Stage(nc, pools, identF, identB, st=1, S=ST1["S"], Q=ST1["Q"],
                       NT=ST1["NT"], tbl=t1m, tw=None, qxyz=q1, p1=p11,
                       out_rows=out1, sum_in=s1in, sum_out=s1out)
            s0.wpall = pools["tbl"].tile([P, s0.n_qt, K], F32, tag="wp0")
            s1.wpall = pools["tbl"].tile([P, s1.n_qt, K], F32, tag="wp1")
            s0.dixa = dix0
            s1.dixa = dix1
            s1.gsrc = tm1x.ap()
            # fill this core's merged-row meta region early
            nc.sync.dma_start(out=p2sx.ap()[:, 0:T1W], in_=t1own.ap()[:, :])

            s0.f1keep = pools["tbl"].tile([P, s0.n_qt, D], F32, tag="f1k0")
            s1.f1keep = pools["tbl"].tile([P, s1.n_qt, D], BF16, tag="f1k1")

            # stage-0 chain (gates the collectives); gpsimd queue order is
            # [s0 gathers, AR0, AllGather, s1 metas, s1 feats, AR1] so no
            # gather ever sits behind a blocking collective it doesn't need.
            s0.emit_tables()
            s1.emit_tables()
            for t in range(s0.n_qt):
                s0.emit_rank(t)
            # early stage-1 rank tiles fill DVE while s0 gathers run
            for t in range(3):
                s1.emit_rank(t)
            for t in range(s0.n_qt):
                s0.emit_meta(t)
                s0.emit_weights(t)
                s0.emit_interp(t)
            s0.emit_allreduce()
            # more rank tiles cover the AllReduce latency ahead of the
            # scal0-gated s0 finalize in the DVE queue
            for t in range(3, 8):
                s1.emit_rank(t)
            for t in range(s0.n_qt):
                s0.emit_finalize(t)
            for t in range(8, 11):
                s1.emit_rank(t)
            nc.gpsimd.collective_compute(
                "AllGather", mybir.AluOpType.bypass, replica_groups=RG,
                ins=[p2sx.ap()], outs=[tm1x.ap()],
            )
            # all merged gathers consecutively on the gpsimd queue; weight
            # tiles staggered into the tail of the rank loop so they drain
            # while top-k still runs (keeping gm slots turning over)
            for t in range(s1.n_qt):
                s1.emit_meta(t)
            w = 0
            for t in range(11, s1.n_qt):
                s1.emit_rank(t)
                s1.emit_weights(w)
                w += 1
            while w < s1.n_qt:
                s1.emit_weights(w)
                w += 1
            for t in range(s1.n_qt):
                s1.emit_interp(t)
            s1.emit_allreduce()
            for t in range(s1.n_qt):
                s1.emit_finalize(t)

    nc.compile()
    _CACHE["nc"] = nc
    return nc


def _pack_bf16(x):
    """[r, n] float -> [r, n//2] float32 words holding bf16 pairs."""
    b = np.ascontiguousarray(x, dtype=np.float32).astype(ml_dtypes.bfloat16)
    return b.view(np.uint16).reshape(x.shape[0], -1).view(np.uint32).view(np.float32)


def _pack(inputs):
    xyz_c = np.ascontiguousarray(inputs["xyz_c"][0], dtype=np.float32)
    xyz_m = np.ascontiguousarray(inputs["xyz_m"][0], dtype=np.float32)
    xyz_f = np.ascontiguousarray(inputs["xyz_f"][0], dtype=np.float32)
    x_c = np.ascontiguousarray(inputs["x_c"][0], dtype=np.float32)
    x_m = np.ascontiguousarray(inputs["x_m"][0], dtype=np.float32)
    x_f = np.ascontiguousarray(inputs["x_f"][0], dtype=np.float32)
    perc_c = np.ascontiguousarray(inputs["perc_c"][0], dtype=np.float32)
    dir_c = np.ascontiguousarray(inputs["dir_c"][0], dtype=np.float32)
    perc_m = np.ascontiguousarray(inputs["perc_m"][0], dtype=np.float32)
    dir_m = np.ascontiguousarray(inputs["dir_m"][0], dtype=np.float32)

    t0 = np.zeros((ST0["S"], T0W), np.float32)
    t0[:, 0:3] = xyz_c
    t0[:, 4:14] = _pack_bf16(perc_c)
    t0[:, 14:44] = _pack_bf16(dir_c.reshape(ST0["S"], 60))
    t0[:, 64:T0W] = _pack_bf16(x_c)

    t1m = np.zeros((ST1["S"], T1W), np.float32)
    t1m[:, 0:3] = xyz_m
    t1m[:, 4:14] = _pack_bf16(perc_m)
    t1m[:, 14:44] = _pack_bf16(dir_m.reshape(ST1["S"], 60))

    in_maps = []
    for c in range(NCORES):
        r0 = slice(c * ST0["Q"], (c + 1) * ST0["Q"])
        r1 = slice(c * ST1["Q"], (c + 1) * ST1["Q"])
        in_maps.append({
            "t0": t0,
            "t1m": t1m,
            "t1own": np.ascontiguousarray(t1m[r0]),
            "q0": np.ascontiguousarray(xyz_m[r0]),
            "q1": np.ascontiguousarray(xyz_f[r1]),
            "p10": _pack_bf16(x_m[r0]),
            "p11": _pack_bf16(x_f[r1]),
        })
    return in_maps


def run_sharded(inputs, trace=False, tmpdir=None):
    """Build + run; returns (full_output, BassKernelResults)."""
    from concourse.bass_utils import run_bass_kernel_spmd
    nc = build()
    in_maps = _pack(inputs)
    res = run_bass_kernel_spmd(nc, in_maps, list(range(NCORES)), trace=trace,
                               tmpdir=tmpdir)
    outs = []
    for c in range(NCORES):
        o = res.results[c]["out1"]  # [Q1, 384] f32 words of bf16 pairs
        outs.append(np.ascontiguousarray(o).view(ml_dtypes.bfloat16)
                    .astype(np.float32).reshape(ST1["Q"], D))
    out = np.concatenate(outs, axis=0)
    return out.reshape(1, ST1["NT"], D), res


def kernel(**inputs) -> np.ndarray:
    out, _ = run_sharded(inputs, trace=False)
    return out
